# revision 33
# baseline (speedup 1.0000x reference)
"""Trainium2 Bass kernel: 1-layer transformer block w/ ALiBi bidirectional attention.

Sharding: data-parallel over batch (B=8) across 8 NeuronCores; zero collectives.

Per-core dataflow (S=1024, D=512, H=8, HD=64, FFN=2048), bf16 matmuls / fp32 PSUM:
  - Host supplies x pre-transposed (xT [d, s]); activations kept natural [s, d]
    for LayerNorm, transposed via PE (identity matmul) where matmuls need it.
  - ALiBi factorization: bias(s,t) = +-slope*(t - s) splits into a per-s term
    (folded into an augmented K=65 row of the q operand; its bf16 rounding is
    constant per softmax column so it cancels exactly in the normalization)
    and a per-t term (the per-partition ACT bias of the fused exp; scores are
    computed transposed: [t partitions, s free]).
  - Each head is half-masked (-1e9) => only the triangular half of the S x S
    score tiles is computed. Diagonal 128x128 tiles are masked by elementwise
    multiply with a 0/1 triangle.
  - q/k are projected per head ([64, 512] PSUM out) so no partition-shifting
    DMAs are needed to split heads.
  - softmax denominator r[s] comes free as an extra output row of the
    probs@V matmul (ones column appended to V); probs@V batches 4 s-chunks
    per PSUM bank; 1/r fused into the PSUM->attn_nat copy.
  - Attention head loop is software-pipelined: scores/exp run two heads
    ahead of probs@V so the PE never drains (avoids HAM re-throttle).
  - LN scale/bias of all three LNs folded into the following weight matrices
    host-side (exact algebra); LN stats batched: one Rsqrt per LN.
"""

import sys

import ml_dtypes
import numpy as np

sys.path.insert(0, "/opt/trn_rl_repo")

import concourse.bass as bass  # noqa: E402,F401
from concourse import bacc  # noqa: E402
import concourse.tile as tile  # noqa: E402
from concourse import mybir  # noqa: E402
from concourse.bass_utils import run_bass_kernel_spmd  # noqa: E402

F32 = mybir.dt.float32
BF16 = mybir.dt.bfloat16
NPBF16 = ml_dtypes.bfloat16
AF = mybir.ActivationFunctionType
OP = mybir.AluOpType

P = 128
B = 8
S = 1024
D = 512
H = 8
HD = 64
FFN = 4 * D
SM = S // P  # 8 sequence chunks
DK = D // P  # 4 feature chunks
FK = FFN // P  # 16 ffn chunks
EPS = 1e-5
N_CORES = 8


def _slopes():
    half = H // 2
    base = 24.0 ** (1.0 / half)
    return (1.0 / base ** np.arange(1, half + 1)).astype(np.float64)


def _fwd(h):
    return h < H // 2


# per (head, j) score-tile geometry for the transposed scores [t=j*128+p, s]
def _s_range(h, j):
    if _fwd(h):  # keep t <= s : s-chunks j..7
        return j * P, S - j * P
    else:  # keep t >= s : s-chunks 0..j
        return 0, (j + 1) * P


def _eoff(h, j):
    off = 0
    for jj in range(j):
        off += _s_range(h, jj)[1]
    return off


def _ewidth(h):
    return _eoff(h, SM - 1) + _s_range(h, SM - 1)[1]  # = 4608


def build_nc(gelu_mode="gelu", zb=False):
    nc = bacc.Bacc("TRN2", target_bir_lowering=False, debug=False)

    def din(name, shape, dt=F32):
        return nc.dram_tensor(name, list(shape), dt, kind="ExternalInput").ap()

    d = {}
    # all big operands pre-tiled host-side: [partition, chunk, free] so every
    # per-partition DMA read is contiguous (full DMA bandwidth)
    d["x"] = din("x", (P, DK, S), BF16)  # pre-transposed host-side
    d["w_in"] = din("w_in", (P, DK, D), BF16)
    d["b_in"] = din("b_in", (D,))
    d["wq"] = din("wq", (P, DK, D), BF16)
    d["wk"] = din("wk", (P, DK, D), BF16)
    d["wv"] = din("wv", (P, DK, D), BF16)
    d["wo"] = din("wo", (P, DK, D), BF16)
    d["bo"] = din("bo", (D,))
    d["w1"] = din("w1", (P, DK, FFN), BF16)
    d["w2"] = din("w2", (P, FK, D), BF16)
    d["b2"] = din("b2", (D,))
    d["w_out"] = din("w_out", (P, DK, D), BF16)
    d["b_out"] = din("b_out", (D,))
    d["bqc"] = din("bqc", (P, DK))
    d["bkc"] = din("bkc", (P, DK))
    d["b1c"] = din("b1c", (P, FK))
    d["bv"] = din("bv", (D,))
    d["qrow"] = din("qrow", (H, S), BF16)
    d["tb"] = din("tb", (P, H * SM))
    d["maskf"] = din("maskf", (P, P), BF16)
    d["maskb"] = din("maskb", (P, P), BF16)
    d["ident"] = din("ident", (P, P), BF16)
    d["out"] = nc.dram_tensor("out", [S, D], F32, kind="ExternalOutput").ap()

    with tile.TileContext(nc) as tc:
        _emit(nc, tc, d, gelu_mode, zb)
    nc.compile()
    return nc


def _emit(nc, tc, d, gelu_mode, zb):
    pool = tc.alloc_tile_pool

    pc = pool(name="consts", bufs=1)
    ph = pool(name="resid", bufs=3)  # tag "h": h1, attn_nat, h2, h3 rotate
    phT = pool(name="transposed", bufs=2)  # tag "hT": xT,hn1T,attnT2,hn2T,hn3T
    psm = pool(name="smalls", bufs=4)
    phn = pool(name="hn_nat", bufs=8)
    pg = pool(name="gelu", bufs=3)
    posb = pool(name="outsb", bufs=3)

    ps_mm = pool(name="ps_mm", bufs=2, space="PSUM")
    ps_acc = pool(name="ps_acc", bufs=4, space="PSUM")
    ps_tr = pool(name="ps_tr", bufs=2, space="PSUM")

    # ---- DMAs in consumption order: x, w_in first (critical path) ----
    xT = phT.tile([P, DK, S], BF16, tag="hT")
    nc.sync.dma_start(out=xT, in_=d["x"])

    pwqkv = pool(name="wqkv", bufs=1)
    win_sb = pwqkv.tile([P, DK, D], BF16, tag="w_in")
    nc.scalar.dma_start(out=win_sb, in_=d["w_in"])

    # small consts next (cheap)
    ident = pc.tile([P, P], BF16, tag="ident")
    nc.sync.dma_start(out=ident, in_=d["ident"])
    maskf = pc.tile([P, P], BF16, tag="maskf")
    nc.sync.dma_start(out=maskf, in_=d["maskf"])
    maskb = pc.tile([P, P], BF16, tag="maskb")
    nc.sync.dma_start(out=maskb, in_=d["maskb"])
    tb = pc.tile([P, H * SM], F32, tag="tb")
    nc.sync.dma_start(out=tb, in_=d["tb"])
    bqc = pc.tile([P, DK], F32, tag="bqc")
    nc.sync.dma_start(out=bqc, in_=d["bqc"])
    bkc = pc.tile([P, DK], F32, tag="bkc")
    nc.sync.dma_start(out=bkc, in_=d["bkc"])
    b1c = pc.tile([P, FK], F32, tag="b1c")
    nc.sync.dma_start(out=b1c, in_=d["b1c"])
    b1cs = pc.tile([P, FK], F32, tag="b1cs")
    nc.vector.tensor_scalar(b1cs, b1c, scalar1=1.702, scalar2=None, op0=OP.mult)

    def bcast(name):
        t = pc.tile([P, D], F32, tag=name + "B")
        nc.gpsimd.dma_start(out=t, in_=d[name].partition_broadcast(P))
        return t

    epsc = pc.tile([P, 1], F32, tag="epsc")
    nc.gpsimd.memset(epsc, EPS)

    binB = bcast("b_in")
    bvB = bcast("bv")
    boB = bcast("bo")
    b2B = bcast("b2")
    boutB = bcast("b_out")

    # remaining weights stream in behind the first-stage ones
    wv_sb = pwqkv.tile([P, DK, D], BF16, tag="wv")
    nc.scalar.dma_start(out=wv_sb, in_=d["wv"])
    wq_sb = pwqkv.tile([P, DK, D], BF16, tag="wq")
    nc.scalar.dma_start(out=wq_sb, in_=d["wq"])
    wk_sb = pwqkv.tile([P, DK, D], BF16, tag="wk")
    nc.scalar.dma_start(out=wk_sb, in_=d["wk"])
    pwo = pool(name="wo_pool", bufs=1)
    wo_sb = pwo.tile([P, DK, D], BF16, tag="wo")
    nc.gpsimd.dma_start(out=wo_sb, in_=d["wo"])
    pwbig = pool(name="wbig", bufs=1)
    w1_sb = pwbig.tile([P, DK, FFN], BF16, tag="w1")
    nc.gpsimd.dma_start(out=w1_sb, in_=d["w1"])
    w2_sb = pwbig.tile([P, FK, D], BF16, tag="w2")
    nc.gpsimd.dma_start(out=w2_sb, in_=d["w2"])
    wout_sb = pwbig.tile([P, DK, D], BF16, tag="w_out")
    nc.gpsimd.dma_start(out=wout_sb, in_=d["w_out"])

    pqk = pool(name="qkheads", bufs=1)
    pva = pool(name="vaug", bufs=1)
    pexp = pool(name="expT", bufs=3)

    def transpose_to(dst, src, eng=0):
        # src [128,128] SBUF -> dst [128,128] (SBUF dest via PSUM bounce)
        t = ps_tr.tile([P, P], BF16, tag="tr")
        nc.tensor.transpose(t, src, ident)
        if eng == 0:
            nc.vector.tensor_copy(dst, t)
        else:
            nc.scalar.copy(dst, t)

    # h1 = x @ w_in + b_in    (natural), skewed with LN1 + v projection
    h1 = ph.tile([P, SM, D], BF16, tag="h")

    # q/k projections: combined dout-pair layout; heads split to qTa/kTa
    # tiles via SBUF->SBUF DMAs spread over two queues, one half at a time
    qTa = {}
    kTa = {}
    for h in range(H):
        qTa[h] = pqk.tile([65, S], BF16, tag=f"qTa{h}", name=f"qTa{h}")
        nc.sync.dma_start(out=qTa[h][64:65, :], in_=d["qrow"][h : h + 1, :])
        kTa[h] = pqk.tile([65, S], BF16, tag=f"kTa{h}", name=f"kTa{h}")
        nc.vector.memset(kTa[h][64:65, :], 1.0)
    pstage = pool(name="stage", bufs=3)
    dma_q = [nc.sync, nc.scalar]
    qno = [0]

    def qk_half(half):
        sl = slice(half * 512, (half + 1) * 512)
        for dd in range(DK):  # head pair (2*dd, 2*dd+1)
            for w_sb, bc, dst in ((wq_sb, bqc, qTa), (wk_sb, bkc, kTa)):
                psq = ps_mm.tile([P, D], F32, tag="mm", name="psq")
                for dk in range(DK):
                    nc.tensor.matmul(
                        psq,
                        w_sb[:, dk, dd * P : (dd + 1) * P],
                        hn1T[:, dk, sl],
                        start=(dk == 0),
                        stop=(dk == DK - 1),
                    )
                stg = pstage.tile([P, D], BF16, tag="stg")
                if zb:
                    nc.scalar.copy(stg, psq)
                else:
                    nc.vector.tensor_scalar(
                        stg, psq, scalar1=bc[:, dd : dd + 1], scalar2=None,
                        op0=OP.add,
                    )
                dma_q[qno[0] % 2].dma_start(
                    out=dst[2 * dd][0:HD, sl], in_=stg[0:HD, :]
                )
                qno[0] += 1
                dma_q[qno[0] % 2].dma_start(
                    out=dst[2 * dd + 1][0:HD, sl], in_=stg[HD:P, :]
                )
                qno[0] += 1

    def h1_m(m):
        ps = ps_mm.tile([P, D], F32, tag="mm")
        for dk in range(DK):
            nc.tensor.matmul(
                ps,
                xT[:, dk, m * P : (m + 1) * P],
                win_sb[:, dk, :],
                start=(dk == 0),
                stop=(dk == DK - 1),
            )
        if zb:
            nc.scalar.copy(h1[:, m, :], ps)
        else:
            nc.vector.tensor_tensor(out=h1[:, m, :], in0=ps, in1=binB, op=OP.add)

    def ln_chain(src, m):
        # LayerNorm scalar chain of chunk m: produces normalized hn tile.
        # hn = (src - mean) * rstd, scale/bias folded into weights host-side
        stats = psm.tile([P, 6], F32, tag="st")
        nc.vector.bn_stats(stats, src[:, m, :])
        mv = psm.tile([P, 2], F32, tag="mv")
        nc.vector.bn_aggr(mv, stats)
        sq = psm.tile([P, 1], F32, tag="sq")
        nc.scalar.activation(sq, mv[:, 1:2], AF.Sqrt, bias=epsc)
        rstd = psm.tile([P, 1], F32, tag="rstd")
        nc.vector.reciprocal(rstd, sq)
        hn = phn.tile([P, D], BF16, tag="hn")
        nc.vector.tensor_scalar(
            hn, src[:, m, :], scalar1=mv[:, 0:1], scalar2=rstd,
            op0=OP.subtract, op1=OP.mult,
        )
        return hn

    def ln_trs(hn, dstT, m):
        for dk in range(DK):
            transpose_to(
                dstT[:, dk, m * P : (m + 1) * P],
                hn[:, dk * P : (dk + 1) * P],
                eng=dk % 2,
            )

    # hn1T = LN1(h1) transposed [d, s]; v right behind its chunk
    hn1T = phT.tile([P, DK, S], BF16, tag="hT")
    v_aug = pva.tile([P, SM, H, 65], BF16, tag="vaug")

    def v_t(t):
        psv = ps_mm.tile([P, D], F32, tag="mm", name="psv")
        for dk in range(DK):
            nc.tensor.matmul(
                psv,
                hn1T[:, dk, t * P : (t + 1) * P],
                wv_sb[:, dk, :],
                start=(dk == 0),
                stop=(dk == DK - 1),
            )
        if zb:
            nc.scalar.copy(
                v_aug[:, t, :, 0:64], psv.rearrange("p (h e) -> p h e", h=H)
            )
        else:
            nc.vector.tensor_tensor(
                out=v_aug[:, t, :, 0:64],
                in0=psv.rearrange("p (h e) -> p h e", h=H),
                in1=bvB.rearrange("p (h e) -> p h e", h=H),
                op=OP.add,
            )
        nc.vector.memset(v_aug[:, t, :, 64:65], 1.0)

    hns = {}
    for m in range(SM):
        h1_m(m)
        hns[m] = ln_chain(h1, m)
    for m in range(SM):
        ln_trs(hns.pop(m), hn1T, m)
        v_t(m)
        if m == 3:
            qk_half(0)
    qk_half(1)


    # ---- attention: software-pipelined over heads ----
    attn_nat = ph.tile([P, SM, D], BF16, tag="h", name="attn_nat")
    expTs = {}

    def stage_scores(h):
        # scores + fused exp + diagonal mask for head h -> expTs[h]
        expT = pexp.tile([P, _ewidth(h)], BF16, tag="expT", name=f"expT{h}")
        expTs[h] = expT
        for j in range(SM):
            s0, w = _s_range(h, j)
            eo = _eoff(h, j)
            off = 0
            while off < w:
                pw = min(512, w - off)
                pss = ps_mm.tile([P, pw], F32, tag="mm", name="pss")
                nc.tensor.matmul(
                    pss,
                    kTa[h][:, j * P : (j + 1) * P],
                    qTa[h][:, s0 + off : s0 + off + pw],
                    start=True,
                    stop=True,
                )
                nc.scalar.activation(
                    expT[:, eo + off : eo + off + pw],
                    pss,
                    AF.Exp,
                    bias=tb[:, h * SM + j : h * SM + j + 1],
                    scale=0.125,
                )
                off += pw
            # mask the diagonal 128x128 block (keep t<=s fwd / t>=s bwd)
            dg = eo if _fwd(h) else eo + j * P
            msk = maskf if _fwd(h) else maskb
            nc.vector.tensor_tensor(
                out=expT[:, dg : dg + P],
                in0=expT[:, dg : dg + P],
                in1=msk,
                op=OP.mult,
            )

    def stage_pv(h):
        # probs @ V with expT stationary (FWL-eligible): output comes out
        # NATURAL [s, dims+denom] - no transpose back needed. 65 rows/matmul.
        expT = expTs[h]
        for half in range(2):
            pvn = ps_acc.tile([P, 4, 66], F32, tag="acc", name=f"pvn_{h}_{half}")
            for mm in range(4):
                m = half * 4 + mm
                js = list(range(0, m + 1)) if _fwd(h) else list(range(m, SM))
                for i, j in enumerate(js):
                    s0, _w = _s_range(h, j)
                    col = _eoff(h, j) + (m * P - s0)
                    nc.tensor.matmul(
                        pvn[:, mm, 0:65],
                        expT[:, col : col + P],
                        v_aug[:, j, h, :],
                        start=(i == 0),
                        stop=(i == len(js) - 1),
                    )
            for mm in range(4):
                m = half * 4 + mm
                rinv = psm.tile([P, 1], F32, tag="rinv")
                nc.vector.reciprocal(rinv, pvn[:, mm, 64:65])
                nc.vector.tensor_scalar(
                    attn_nat[:, m, h * HD : (h + 1) * HD],
                    pvn[:, mm, 0:64],
                    scalar1=rinv,
                    scalar2=None,
                    op0=OP.mult,
                )

    stage_scores(0)
    stage_scores(1)
    for h in range(H):
        if h + 2 < H:
            stage_scores(h + 2)
        stage_pv(h)

    # attn transpose + wo + residual + LN2, pipelined per m-chunk
    attnT2 = phT.tile([P, DK, S], BF16, tag="hT")
    h2 = ph.tile([P, SM, D], BF16, tag="h")
    hn2T = phT.tile([P, DK, S], BF16, tag="hT")

    def wo_m(m):
        for dk in range(DK):
            transpose_to(
                attnT2[:, dk, m * P : (m + 1) * P],
                attn_nat[:, m, dk * P : (dk + 1) * P],
                eng=dk % 2,
            )
        ps = ps_mm.tile([P, D], F32, tag="mm", name="pswo")
        for dk in range(DK):
            nc.tensor.matmul(
                ps,
                attnT2[:, dk, m * P : (m + 1) * P],
                wo_sb[:, dk, :],
                start=(dk == 0),
                stop=(dk == DK - 1),
            )
        if zb:
            nc.vector.tensor_tensor(
                out=h2[:, m, :], in0=ps, in1=h1[:, m, :], op=OP.add
            )
        else:
            nc.vector.tensor_tensor(out=h2[:, m, :], in0=ps, in1=boB, op=OP.add)
            nc.vector.tensor_tensor(
                out=h2[:, m, :], in0=h2[:, m, :], in1=h1[:, m, :], op=OP.add
            )

    hns2 = {}
    for m in range(SM):
        wo_m(m)
        if m >= 1:
            hns2[m - 1] = ln_chain(h2, m - 1)
    hns2[SM - 1] = ln_chain(h2, SM - 1)
    for m in range(SM):
        ln_trs(hns2.pop(m), hn2T, m)

    # ---- FFN: h3 = h2 + gelu(hn2 @ w1 + b1) @ w2 + b2 ----
    # final LN + out projection run per-chunk inside the FFN tail so the
    # second half's GEMMs overlap the first half's LN3/out chain
    h3 = ph.tile([P, SM, D], BF16, tag="h")
    hn3T = phT.tile([P, DK, S], BF16, tag="hT")
    hns3 = {}
    out_view = d["out"].rearrange("(c p) n -> p c n", p=P)

    def out_m(m):
        ps = ps_mm.tile([P, D], F32, tag="mm", name="psout")
        for dk in range(DK):
            nc.tensor.matmul(
                ps,
                hn3T[:, dk, m * P : (m + 1) * P],
                wout_sb[:, dk, :],
                start=(dk == 0),
                stop=(dk == DK - 1),
            )
        osb = posb.tile([P, D], F32, tag="osb")
        if zb:
            nc.scalar.copy(osb, ps)
        else:
            nc.vector.tensor_tensor(out=osb, in0=ps, in1=boutB, op=OP.add)
        eng = (nc.sync, nc.scalar, nc.gpsimd)[m % 3]
        eng.dma_start(out=out_view[:, m, :], in_=osb)
    # 4 quarters of 2 s-chunks each: quarter q's LN3 + out projection run
    # during quarter q+1's GEMMs, shrinking the serial tail to one quarter
    for q in range(4):
        accs = []
        for mm in range(2):
            accs.append(
                ps_acc.tile([P, D], F32, tag="acc", name=f"ff2ps{q}_{mm}")
            )
        for kc in range(FK):
            if q >= 1 and kc == 8:
                for m in (2 * (q - 1), 2 * (q - 1) + 1):
                    ln_trs(hns3.pop(m), hn3T, m)
                    out_m(m)
            ps1 = ps_mm.tile([P, 256], F32, tag="mm", name="ff1ps")
            for dk in range(DK):
                nc.tensor.matmul(
                    ps1,
                    w1_sb[:, dk, kc * P : (kc + 1) * P],
                    hn2T[:, dk, q * 256 : (q + 1) * 256],
                    start=(dk == 0),
                    stop=(dk == DK - 1),
                )
            gt = pg.tile([P, 256], BF16, tag="gt")
            if gelu_mode == "gelu":
                nc.scalar.activation(gt, ps1, AF.Gelu, bias=b1c[:, kc : kc + 1])
            else:  # CoreSim lacks Gelu: x*sigmoid(1.702x) stand-in
                sg = pg.tile([P, 256], BF16, tag="sg")
                nc.scalar.activation(
                    sg, ps1, AF.Sigmoid, bias=b1cs[:, kc : kc + 1], scale=1.702
                )
                xb = pg.tile([P, 256], BF16, tag="xb")
                nc.vector.tensor_scalar(
                    xb, ps1, scalar1=b1c[:, kc : kc + 1], scalar2=None, op0=OP.add
                )
                nc.vector.tensor_tensor(out=gt, in0=sg, in1=xb, op=OP.mult)
            for mm in range(2):
                nc.tensor.matmul(
                    accs[mm],
                    gt[:, mm * P : (mm + 1) * P],
                    w2_sb[:, kc, :],
                    start=(kc == 0),
                    stop=(kc == FK - 1),
                )
        for mm in range(2):
            m = q * 2 + mm
            if zb:
                nc.vector.tensor_tensor(
                    out=h3[:, m, :], in0=accs[mm], in1=h2[:, m, :], op=OP.add
                )
            else:
                nc.vector.tensor_tensor(
                    out=h3[:, m, :], in0=accs[mm], in1=b2B, op=OP.add
                )
                nc.vector.tensor_tensor(
                    out=h3[:, m, :], in0=h3[:, m, :], in1=h2[:, m, :], op=OP.add
                )
            hns3[m] = ln_chain(h3, m)
    for m in (SM - 2, SM - 1):
        ln_trs(hns3.pop(m), hn3T, m)
        out_m(m)

    for p_ in (pstage, pexp, pva, pqk, pwbig, pwo, pwqkv, ps_tr, ps_acc,
               ps_mm, posb, pg, phn, psm, phT, ph, pc):
        p_.release()


def host_prep(inputs):
    """Fold LN affine params into weights; build ALiBi helper tensors."""
    f = lambda k: np.asarray(inputs[k], dtype=np.float64)
    ln1_s, ln1_b = f("ln1_s"), f("ln1_b")
    ln2_s, ln2_b = f("ln2_s"), f("ln2_b")
    lnf_s, lnf_b = f("lnf_s"), f("lnf_b")
    wq, bq = f("wq"), f("bq")
    wk, bk = f("wk"), f("bk")
    wv, bv = f("wv"), f("bv")
    w1, b1 = f("w1"), f("b1")
    w_out, b_out = f("w_out"), f("b_out")

    wq_f = (ln1_s[:, None] * wq).astype(np.float32)
    bq_f = (bq + ln1_b @ wq).astype(np.float32)
    wk_f = (ln1_s[:, None] * wk).astype(np.float32)
    bk_f = (bk + ln1_b @ wk).astype(np.float32)
    wv_f = (ln1_s[:, None] * wv).astype(np.float32)
    bv_f = (bv + ln1_b @ wv).astype(np.float32)
    w1_f = (ln2_s[:, None] * w1).astype(np.float32)
    b1_f = (b1 + ln2_b @ w1).astype(np.float32)
    wout_f = (lnf_s[:, None] * w_out).astype(np.float32)
    bout_f = (b_out + lnf_b @ w_out).astype(np.float32)

    sl = _slopes()
    qrow = np.zeros((H, S), np.float32)
    tb = np.zeros((P, H * SM), np.float32)
    s_idx = np.arange(S, dtype=np.float64)
    p_idx = np.arange(P, dtype=np.float64)
    for h in range(H):
        sgn = -1.0 if h < H // 2 else 1.0  # sign of the per-s row term
        qrow[h] = (sgn * 8.0 * sl[h % 4] * s_idx).astype(np.float32)
        for j in range(SM):
            tb[:, h * SM + j] = (-sgn * sl[h % 4] * (j * P + p_idx)).astype(
                np.float32
            )
    maskf = np.triu(np.ones((P, P), np.float32))  # keep t <= s (p <= c)
    maskb = np.tril(np.ones((P, P), np.float32))  # keep t >= s (p >= c)

    def tile3(arr):
        # [C*P, N] -> [P, C, N] contiguous (pre-tiled for SBUF layout)
        cp, n = arr.shape
        return np.ascontiguousarray(
            arr.reshape(cp // P, P, n).transpose(1, 0, 2)
        )

    common = {
        "w_in": tile3(np.asarray(inputs["w_in"], np.float32).astype(NPBF16)),
        "b_in": np.asarray(inputs["b_in"], np.float32),
        "wq": tile3(wq_f.astype(NPBF16)),
        "wk": tile3(wk_f.astype(NPBF16)),
        "wv": tile3(wv_f.astype(NPBF16)),
        "wo": tile3(np.asarray(inputs["wo"], np.float32).astype(NPBF16)),
        "bo": np.asarray(inputs["bo"], np.float32),
        "w1": tile3(w1_f.astype(NPBF16)),
        "w2": tile3(np.asarray(inputs["w2"], np.float32).astype(NPBF16)),
        "b2": np.asarray(inputs["b2"], np.float32),
        "w_out": tile3(wout_f.astype(NPBF16)),
        "b_out": bout_f,
        "bqc": np.ascontiguousarray(bq_f.reshape(DK, P).T),
        "bkc": np.ascontiguousarray(bk_f.reshape(DK, P).T),
        "b1c": np.ascontiguousarray(b1_f.reshape(FK, P).T),
        "bv": bv_f,
        "qrow": qrow.astype(NPBF16),
        "tb": tb,
        "maskf": maskf.astype(NPBF16),
        "maskb": maskb.astype(NPBF16),
        "ident": np.eye(P, dtype=NPBF16),
    }
    return common


_NC_CACHE = {}


def get_nc(gelu_mode="gelu", zb=False):
    key = (gelu_mode, zb)
    if key not in _NC_CACHE:
        _NC_CACHE[key] = build_nc(gelu_mode, zb)
    return _NC_CACHE[key]


def _all_zero_biases(common):
    names = ["b_in", "bo", "b2", "b_out", "bv"]
    if any(np.any(np.asarray(common[k], np.float32)) for k in names):
        return False
    if np.any(common["bqc"]) or np.any(common["bkc"]):
        return False
    return True


def run(inputs, trace=False):
    common = host_prep(inputs)
    x = np.asarray(inputs["x"], np.float32)
    def xtile(xi):
        # [S, D] -> xT [D, S] -> [P, DK, S] contiguous
        xt = xi.T.astype(NPBF16)
        return np.ascontiguousarray(
            xt.reshape(DK, P, S).transpose(1, 0, 2)
        )

    in_maps = [dict(common, x=xtile(x[i])) for i in range(N_CORES)]
    nc = get_nc(zb=_all_zero_biases(common))
    res = run_bass_kernel_spmd(
        nc, in_maps, core_ids=list(range(N_CORES)), trace=trace
    )
    out = np.stack([res.results[i]["out"] for i in range(N_CORES)])
    return out.astype(np.float32), res


def kernel(**inputs):
    out, _ = run(inputs, trace=False)
    return out


# revision 35
# speedup vs baseline: 1.1072x; 1.1072x over previous
"""Trainium2 Bass kernel: 1-layer transformer block w/ ALiBi bidirectional attention.

Sharding: data-parallel over batch (B=8) across 8 NeuronCores; zero collectives.

Per-core dataflow (S=1024, D=512, H=8, HD=64, FFN=2048), bf16 matmuls / fp32 PSUM:
  - Host supplies x pre-transposed (xT [d, s]); activations kept natural [s, d]
    for LayerNorm, transposed via PE (identity matmul) where matmuls need it.
  - ALiBi factorization: bias(s,t) = +-slope*(t - s) splits into a per-s term
    (folded into an augmented K=65 row of the q operand; its bf16 rounding is
    constant per softmax column so it cancels exactly in the normalization)
    and a per-t term (the per-partition ACT bias of the fused exp; scores are
    computed transposed: [t partitions, s free]).
  - Each head is half-masked (-1e9) => only the triangular half of the S x S
    score tiles is computed. Diagonal 128x128 tiles are masked by elementwise
    multiply with a 0/1 triangle.
  - q/k are projected per head ([64, 512] PSUM out) so no partition-shifting
    DMAs are needed to split heads.
  - softmax denominator r[s] comes free as an extra output row of the
    probs@V matmul (ones column appended to V); probs@V batches 4 s-chunks
    per PSUM bank; 1/r fused into the PSUM->attn_nat copy.
  - Attention head loop is software-pipelined: scores/exp run two heads
    ahead of probs@V so the PE never drains (avoids HAM re-throttle).
  - LN scale/bias of all three LNs folded into the following weight matrices
    host-side (exact algebra); LN stats batched: one Rsqrt per LN.
"""

import sys

import ml_dtypes
import numpy as np

sys.path.insert(0, "/opt/trn_rl_repo")

import concourse.bass as bass  # noqa: E402,F401
from concourse import bacc  # noqa: E402
import concourse.tile as tile  # noqa: E402
from concourse import mybir  # noqa: E402
from concourse.bass_utils import run_bass_kernel_spmd  # noqa: E402

F32 = mybir.dt.float32
BF16 = mybir.dt.bfloat16
NPBF16 = ml_dtypes.bfloat16
AF = mybir.ActivationFunctionType
OP = mybir.AluOpType

P = 128
B = 8
S = 1024
D = 512
H = 8
HD = 64
FFN = 4 * D
SM = S // P  # 8 sequence chunks
DK = D // P  # 4 feature chunks
FK = FFN // P  # 16 ffn chunks
EPS = 1e-5
N_CORES = 8


def _slopes():
    half = H // 2
    base = 24.0 ** (1.0 / half)
    return (1.0 / base ** np.arange(1, half + 1)).astype(np.float64)


def _fwd(h):
    return h < H // 2


# per (head, j) score-tile geometry for the transposed scores [t=j*128+p, s]
def _s_range(h, j):
    if _fwd(h):  # keep t <= s : s-chunks j..7
        return j * P, S - j * P
    else:  # keep t >= s : s-chunks 0..j
        return 0, (j + 1) * P


def _eoff(h, j):
    off = 0
    for jj in range(j):
        off += _s_range(h, jj)[1]
    return off


def _ewidth(h):
    return _eoff(h, SM - 1) + _s_range(h, SM - 1)[1]  # = 4608


def build_nc(gelu_mode="gelu", zb=False):
    nc = bacc.Bacc("TRN2", target_bir_lowering=False, debug=False)

    def din(name, shape, dt=F32):
        return nc.dram_tensor(name, list(shape), dt, kind="ExternalInput").ap()

    d = {}
    # all big operands pre-tiled host-side: [partition, chunk, free] so every
    # per-partition DMA read is contiguous (full DMA bandwidth)
    d["x"] = din("x", (P, DK, S), BF16)  # pre-transposed host-side
    d["w_in"] = din("w_in", (P, DK, D), BF16)
    d["b_in"] = din("b_in", (D,))
    d["wq"] = din("wq", (P, DK, D), BF16)
    d["wk"] = din("wk", (P, DK, D), BF16)
    d["wv"] = din("wv", (P, DK, D), BF16)
    d["wo"] = din("wo", (P, DK, D), BF16)
    d["bo"] = din("bo", (D,))
    d["w1"] = din("w1", (P, DK, FFN), BF16)
    d["w2"] = din("w2", (P, FK, D), BF16)
    d["b2"] = din("b2", (D,))
    d["w_out"] = din("w_out", (P, DK, D), BF16)
    d["b_out"] = din("b_out", (D,))
    d["bqc"] = din("bqc", (P, DK))
    d["bkc"] = din("bkc", (P, DK))
    d["b1c"] = din("b1c", (P, FK))
    d["bv"] = din("bv", (D,))
    d["qrow"] = din("qrow", (H, S), BF16)
    d["tb"] = din("tb", (P, H * SM))
    d["maskf"] = din("maskf", (P, P), BF16)
    d["maskb"] = din("maskb", (P, P), BF16)
    d["ident"] = din("ident", (P, P), BF16)
    d["out"] = nc.dram_tensor("out", [S, D], F32, kind="ExternalOutput").ap()

    with tile.TileContext(nc) as tc:
        _emit(nc, tc, d, gelu_mode, zb)
    nc.compile()
    return nc


def _emit(nc, tc, d, gelu_mode, zb):
    pool = tc.alloc_tile_pool

    pc = pool(name="consts", bufs=1)
    ph = pool(name="resid", bufs=3)  # tag "h": h1, attn_nat, h2, h3 rotate
    phT = pool(name="transposed", bufs=2)  # tag "hT": xT,hn1T,attnT2,hn2T,hn3T
    psm = pool(name="smalls", bufs=4)
    phn = pool(name="hn_nat", bufs=8)
    pg = pool(name="gelu", bufs=3)
    posb = pool(name="outsb", bufs=3)

    ps_mm = pool(name="ps_mm", bufs=2, space="PSUM")
    ps_acc = pool(name="ps_acc", bufs=4, space="PSUM")
    ps_tr = pool(name="ps_tr", bufs=2, space="PSUM")

    # ---- DMAs in consumption order: x, w_in first (critical path) ----
    xT = phT.tile([P, DK, S], BF16, tag="hT")
    nc.sync.dma_start(out=xT[:, 0:2, :], in_=d["x"][:, 0:2, :])
    nc.scalar.dma_start(out=xT[:, 2:4, :], in_=d["x"][:, 2:4, :])

    pwqkv = pool(name="wqkv", bufs=1)
    win_sb = pwqkv.tile([P, DK, D], BF16, tag="w_in")
    nc.scalar.dma_start(out=win_sb, in_=d["w_in"])

    # small consts on the gpsimd queue, ahead of the big weights (fast)
    ident = pc.tile([P, P], BF16, tag="ident")
    nc.gpsimd.dma_start(out=ident, in_=d["ident"])
    maskf = pc.tile([P, P], BF16, tag="maskf")
    nc.gpsimd.dma_start(out=maskf, in_=d["maskf"])
    maskb = pc.tile([P, P], BF16, tag="maskb")
    nc.gpsimd.dma_start(out=maskb, in_=d["maskb"])
    tb = pc.tile([P, H * SM], F32, tag="tb")
    nc.gpsimd.dma_start(out=tb, in_=d["tb"])
    b1c = pc.tile([P, FK], F32, tag="b1c")
    nc.gpsimd.dma_start(out=b1c, in_=d["b1c"])
    qrowc = pc.tile([1, H, S], BF16, tag="qrowc")
    nc.gpsimd.dma_start(out=qrowc, in_=d["qrow"].rearrange("h s -> (h s)").unsqueeze(0).rearrange("o (h s) -> o h s", h=H))

    epsc = pc.tile([P, 1], F32, tag="epsc")
    nc.vector.memset(epsc, EPS)

    if gelu_mode != "gelu":
        b1cs = pc.tile([P, FK], F32, tag="b1cs")
        nc.vector.tensor_scalar(
            b1cs, b1c, scalar1=1.702, scalar2=None, op0=OP.mult
        )

    def bcast(name):
        t = pc.tile([P, D], F32, tag=name + "B")
        nc.gpsimd.dma_start(out=t, in_=d[name].partition_broadcast(P))
        return t

    if not zb:
        bqc = pc.tile([P, DK], F32, tag="bqc")
        nc.gpsimd.dma_start(out=bqc, in_=d["bqc"])
        bkc = pc.tile([P, DK], F32, tag="bkc")
        nc.gpsimd.dma_start(out=bkc, in_=d["bkc"])
        binB = bcast("b_in")
        bvB = bcast("bv")
        boB = bcast("bo")
        b2B = bcast("b2")
        boutB = bcast("b_out")
    else:
        bqc = bkc = None

    # remaining weights stream in behind the first-stage ones
    wv_sb = pwqkv.tile([P, DK, D], BF16, tag="wv")
    nc.scalar.dma_start(out=wv_sb, in_=d["wv"])
    wq_sb = pwqkv.tile([P, DK, D], BF16, tag="wq")
    nc.scalar.dma_start(out=wq_sb, in_=d["wq"])
    wk_sb = pwqkv.tile([P, DK, D], BF16, tag="wk")
    nc.scalar.dma_start(out=wk_sb, in_=d["wk"])
    pwo = pool(name="wo_pool", bufs=1)
    wo_sb = pwo.tile([P, DK, D], BF16, tag="wo")
    nc.gpsimd.dma_start(out=wo_sb, in_=d["wo"])
    pwbig = pool(name="wbig", bufs=1)
    w1_sb = pwbig.tile([P, DK, FFN], BF16, tag="w1")
    nc.gpsimd.dma_start(out=w1_sb, in_=d["w1"])
    w2_sb = pwbig.tile([P, FK, D], BF16, tag="w2")
    nc.gpsimd.dma_start(out=w2_sb, in_=d["w2"])
    wout_sb = pwbig.tile([P, DK, D], BF16, tag="w_out")
    nc.gpsimd.dma_start(out=wout_sb, in_=d["w_out"])

    pqk = pool(name="qkheads", bufs=1)
    pva = pool(name="vaug", bufs=1)
    pexp = pool(name="expT", bufs=3)

    def transpose_to(dst, src, eng=0):
        # src [128,128] SBUF -> dst [128,128] (SBUF dest via PSUM bounce)
        t = ps_tr.tile([P, P], BF16, tag="tr")
        nc.tensor.transpose(t, src, ident)
        if eng == 0:
            nc.vector.tensor_copy(dst, t)
        else:
            nc.scalar.copy(dst, t)

    # h1 = x @ w_in + b_in    (natural), skewed with LN1 + v projection
    h1 = ph.tile([P, SM, D], BF16, tag="h")

    # q/k projections: combined dout-pair layout; heads split to qTa/kTa
    # tiles via SBUF->SBUF DMAs spread over two queues, one half at a time
    qTa_t = pqk.tile([65, H, S], BF16, tag="qTa", name="qTa_t")
    nc.vector.tensor_copy(qTa_t[64:65, :, :], qrowc)
    kTa_t = pqk.tile([65, H, S], BF16, tag="kTa", name="kTa_t")
    nc.vector.memset(kTa_t[64:65, :, :], 1.0)
    qTa = {h: qTa_t[:, h, :] for h in range(H)}
    kTa = {h: kTa_t[:, h, :] for h in range(H)}
    pstage = pool(name="stage", bufs=3)
    dma_q = [nc.sync, nc.scalar]
    qno = [0]

    def qk_half(half):
        sl = slice(half * 512, (half + 1) * 512)
        for dd in range(DK):  # head pair (2*dd, 2*dd+1)
            for w_sb, bc, dst in ((wq_sb, bqc, qTa), (wk_sb, bkc, kTa)):
                psq = ps_mm.tile([P, D], F32, tag="mm", name="psq")
                for dk in range(DK):
                    nc.tensor.matmul(
                        psq,
                        w_sb[:, dk, dd * P : (dd + 1) * P],
                        hn1T[:, dk, sl],
                        start=(dk == 0),
                        stop=(dk == DK - 1),
                    )
                stg = pstage.tile([P, D], BF16, tag="stg")
                if zb:
                    nc.scalar.copy(stg, psq)
                else:
                    nc.vector.tensor_scalar(
                        stg, psq, scalar1=bc[:, dd : dd + 1], scalar2=None,
                        op0=OP.add,
                    )
                dma_q[qno[0] % 2].dma_start(
                    out=dst[2 * dd][0:HD, sl], in_=stg[0:HD, :]
                )
                qno[0] += 1
                dma_q[qno[0] % 2].dma_start(
                    out=dst[2 * dd + 1][0:HD, sl], in_=stg[HD:P, :]
                )
                qno[0] += 1

    def h1_m(m):
        ps = ps_mm.tile([P, D], F32, tag="mm")
        for dk in range(DK):
            nc.tensor.matmul(
                ps,
                xT[:, dk, m * P : (m + 1) * P],
                win_sb[:, dk, :],
                start=(dk == 0),
                stop=(dk == DK - 1),
            )
        if zb:
            nc.scalar.copy(h1[:, m, :], ps)
        else:
            nc.vector.tensor_tensor(out=h1[:, m, :], in0=ps, in1=binB, op=OP.add)

    def ln_chain(src, m):
        # LayerNorm scalar chain of chunk m: produces normalized hn tile.
        # hn = (src - mean) * rstd, scale/bias folded into weights host-side
        stats = psm.tile([P, 6], F32, tag="st")
        nc.vector.bn_stats(stats, src[:, m, :])
        mv = psm.tile([P, 2], F32, tag="mv")
        nc.vector.bn_aggr(mv, stats)
        sq = psm.tile([P, 1], F32, tag="sq")
        nc.scalar.activation(sq, mv[:, 1:2], AF.Sqrt, bias=epsc)
        rstd = psm.tile([P, 1], F32, tag="rstd")
        nc.vector.reciprocal(rstd, sq)
        hn = phn.tile([P, D], BF16, tag="hn")
        nc.vector.tensor_scalar(
            hn, src[:, m, :], scalar1=mv[:, 0:1], scalar2=rstd,
            op0=OP.subtract, op1=OP.mult,
        )
        return hn

    def ln_trs(hn, dstT, m):
        for dk in range(DK):
            transpose_to(
                dstT[:, dk, m * P : (m + 1) * P],
                hn[:, dk * P : (dk + 1) * P],
                eng=dk % 2,
            )

    # hn1T = LN1(h1) transposed [d, s]; v right behind its chunk
    hn1T = phT.tile([P, DK, S], BF16, tag="hT")
    v_aug = pva.tile([P, SM, H, 65], BF16, tag="vaug")

    def v_t(t):
        psv = ps_mm.tile([P, D], F32, tag="mm", name="psv")
        for dk in range(DK):
            nc.tensor.matmul(
                psv,
                hn1T[:, dk, t * P : (t + 1) * P],
                wv_sb[:, dk, :],
                start=(dk == 0),
                stop=(dk == DK - 1),
            )
        if zb:
            nc.scalar.copy(
                v_aug[:, t, :, 0:64], psv.rearrange("p (h e) -> p h e", h=H)
            )
        else:
            nc.vector.tensor_tensor(
                out=v_aug[:, t, :, 0:64],
                in0=psv.rearrange("p (h e) -> p h e", h=H),
                in1=bvB.rearrange("p (h e) -> p h e", h=H),
                op=OP.add,
            )
        nc.vector.memset(v_aug[:, t, :, 64:65], 1.0)

    hns = {}
    for m in range(SM):
        h1_m(m)
        hns[m] = ln_chain(h1, m)
    for m in range(SM):
        ln_trs(hns.pop(m), hn1T, m)
        v_t(m)
        if m == 3:
            qk_half(0)
    qk_half(1)


    # ---- attention: software-pipelined over heads ----
    attn_nat = ph.tile([P, SM, D], BF16, tag="h", name="attn_nat")
    expTs = {}

    def stage_scores(h):
        # scores + fused exp + diagonal mask for head h -> expTs[h]
        expT = pexp.tile([P, _ewidth(h)], BF16, tag="expT", name=f"expT{h}")
        expTs[h] = expT
        for j in range(SM):
            s0, w = _s_range(h, j)
            eo = _eoff(h, j)
            off = 0
            while off < w:
                pw = min(512, w - off)
                pss = ps_mm.tile([P, pw], F32, tag="mm", name="pss")
                nc.tensor.matmul(
                    pss,
                    kTa[h][:, j * P : (j + 1) * P],
                    qTa[h][:, s0 + off : s0 + off + pw],
                    start=True,
                    stop=True,
                )
                nc.scalar.activation(
                    expT[:, eo + off : eo + off + pw],
                    pss,
                    AF.Exp,
                    bias=tb[:, h * SM + j : h * SM + j + 1],
                    scale=0.125,
                )
                off += pw
            # mask the diagonal 128x128 block (keep t<=s fwd / t>=s bwd)
            dg = eo if _fwd(h) else eo + j * P
            msk = maskf if _fwd(h) else maskb
            nc.vector.tensor_tensor(
                out=expT[:, dg : dg + P],
                in0=expT[:, dg : dg + P],
                in1=msk,
                op=OP.mult,
            )

    def stage_pv(h):
        # probs @ V with expT stationary (FWL-eligible): output comes out
        # NATURAL [s, dims+denom] - no transpose back needed. 65 rows/matmul.
        expT = expTs[h]
        for half in range(2):
            pvn = ps_acc.tile([P, 4, 66], F32, tag="acc", name=f"pvn_{h}_{half}")
            for mm in range(4):
                m = half * 4 + mm
                js = list(range(0, m + 1)) if _fwd(h) else list(range(m, SM))
                for i, j in enumerate(js):
                    s0, _w = _s_range(h, j)
                    col = _eoff(h, j) + (m * P - s0)
                    nc.tensor.matmul(
                        pvn[:, mm, 0:65],
                        expT[:, col : col + P],
                        v_aug[:, j, h, :],
                        start=(i == 0),
                        stop=(i == len(js) - 1),
                    )
            for mm in range(4):
                m = half * 4 + mm
                rinv = psm.tile([P, 1], F32, tag="rinv")
                nc.vector.reciprocal(rinv, pvn[:, mm, 64:65])
                nc.vector.tensor_scalar(
                    attn_nat[:, m, h * HD : (h + 1) * HD],
                    pvn[:, mm, 0:64],
                    scalar1=rinv,
                    scalar2=None,
                    op0=OP.mult,
                )

    stage_scores(0)
    stage_scores(1)
    for h in range(H):
        if h + 2 < H:
            stage_scores(h + 2)
        stage_pv(h)

    # attn transpose + wo + residual + LN2, pipelined per m-chunk
    attnT2 = phT.tile([P, DK, S], BF16, tag="hT")
    h2 = ph.tile([P, SM, D], BF16, tag="h")
    hn2T = phT.tile([P, DK, S], BF16, tag="hT")

    def wo_m(m):
        for dk in range(DK):
            transpose_to(
                attnT2[:, dk, m * P : (m + 1) * P],
                attn_nat[:, m, dk * P : (dk + 1) * P],
                eng=dk % 2,
            )
        ps = ps_mm.tile([P, D], F32, tag="mm", name="pswo")
        for dk in range(DK):
            nc.tensor.matmul(
                ps,
                attnT2[:, dk, m * P : (m + 1) * P],
                wo_sb[:, dk, :],
                start=(dk == 0),
                stop=(dk == DK - 1),
            )
        if zb:
            nc.vector.tensor_tensor(
                out=h2[:, m, :], in0=ps, in1=h1[:, m, :], op=OP.add
            )
        else:
            nc.vector.tensor_tensor(out=h2[:, m, :], in0=ps, in1=boB, op=OP.add)
            nc.vector.tensor_tensor(
                out=h2[:, m, :], in0=h2[:, m, :], in1=h1[:, m, :], op=OP.add
            )

    hns2 = {}
    for m in range(SM):
        wo_m(m)
        if m >= 1:
            hns2[m - 1] = ln_chain(h2, m - 1)
    hns2[SM - 1] = ln_chain(h2, SM - 1)
    for m in range(SM):
        ln_trs(hns2.pop(m), hn2T, m)

    # ---- FFN: h3 = h2 + gelu(hn2 @ w1 + b1) @ w2 + b2 ----
    # final LN + out projection run per-chunk inside the FFN tail so the
    # second half's GEMMs overlap the first half's LN3/out chain
    h3 = ph.tile([P, SM, D], BF16, tag="h")
    hn3T = phT.tile([P, DK, S], BF16, tag="hT")
    hns3 = {}
    out_view = d["out"].rearrange("(c p) n -> p c n", p=P)

    def out_m(m):
        ps = ps_mm.tile([P, D], F32, tag="mm", name="psout")
        for dk in range(DK):
            nc.tensor.matmul(
                ps,
                hn3T[:, dk, m * P : (m + 1) * P],
                wout_sb[:, dk, :],
                start=(dk == 0),
                stop=(dk == DK - 1),
            )
        osb = posb.tile([P, D], F32, tag="osb")
        if zb:
            nc.scalar.copy(osb, ps)
        else:
            nc.vector.tensor_tensor(out=osb, in0=ps, in1=boutB, op=OP.add)
        eng = (nc.sync, nc.scalar, nc.gpsimd)[m % 3]
        eng.dma_start(out=out_view[:, m, :], in_=osb)
    # 4 quarters of 2 s-chunks each: quarter q's LN3 + out projection run
    # during quarter q+1's GEMMs, shrinking the serial tail to one quarter
    for q in range(4):
        accs = []
        for mm in range(2):
            accs.append(
                ps_acc.tile([P, D], F32, tag="acc", name=f"ff2ps{q}_{mm}")
            )
        for kc in range(FK):
            if q >= 1 and kc == 8:
                for m in (2 * (q - 1), 2 * (q - 1) + 1):
                    ln_trs(hns3.pop(m), hn3T, m)
                    out_m(m)
            ps1 = ps_mm.tile([P, 256], F32, tag="mm", name="ff1ps")
            for dk in range(DK):
                nc.tensor.matmul(
                    ps1,
                    w1_sb[:, dk, kc * P : (kc + 1) * P],
                    hn2T[:, dk, q * 256 : (q + 1) * 256],
                    start=(dk == 0),
                    stop=(dk == DK - 1),
                )
            gt = pg.tile([P, 256], BF16, tag="gt")
            if gelu_mode == "gelu":
                nc.scalar.activation(gt, ps1, AF.Gelu, bias=b1c[:, kc : kc + 1])
            else:  # CoreSim lacks Gelu: x*sigmoid(1.702x) stand-in
                sg = pg.tile([P, 256], BF16, tag="sg")
                nc.scalar.activation(
                    sg, ps1, AF.Sigmoid, bias=b1cs[:, kc : kc + 1], scale=1.702
                )
                xb = pg.tile([P, 256], BF16, tag="xb")
                nc.vector.tensor_scalar(
                    xb, ps1, scalar1=b1c[:, kc : kc + 1], scalar2=None, op0=OP.add
                )
                nc.vector.tensor_tensor(out=gt, in0=sg, in1=xb, op=OP.mult)
            for mm in range(2):
                nc.tensor.matmul(
                    accs[mm],
                    gt[:, mm * P : (mm + 1) * P],
                    w2_sb[:, kc, :],
                    start=(kc == 0),
                    stop=(kc == FK - 1),
                )
        for mm in range(2):
            m = q * 2 + mm
            if zb:
                nc.vector.tensor_tensor(
                    out=h3[:, m, :], in0=accs[mm], in1=h2[:, m, :], op=OP.add
                )
            else:
                nc.vector.tensor_tensor(
                    out=h3[:, m, :], in0=accs[mm], in1=b2B, op=OP.add
                )
                nc.vector.tensor_tensor(
                    out=h3[:, m, :], in0=h3[:, m, :], in1=h2[:, m, :], op=OP.add
                )
            hns3[m] = ln_chain(h3, m)
    for m in (SM - 2, SM - 1):
        ln_trs(hns3.pop(m), hn3T, m)
        out_m(m)

    for p_ in (pstage, pexp, pva, pqk, pwbig, pwo, pwqkv, ps_tr, ps_acc,
               ps_mm, posb, pg, phn, psm, phT, ph, pc):
        p_.release()


def host_prep(inputs):
    """Fold LN affine params into weights; build ALiBi helper tensors."""
    f = lambda k: np.asarray(inputs[k], dtype=np.float64)
    ln1_s, ln1_b = f("ln1_s"), f("ln1_b")
    ln2_s, ln2_b = f("ln2_s"), f("ln2_b")
    lnf_s, lnf_b = f("lnf_s"), f("lnf_b")
    wq, bq = f("wq"), f("bq")
    wk, bk = f("wk"), f("bk")
    wv, bv = f("wv"), f("bv")
    w1, b1 = f("w1"), f("b1")
    w_out, b_out = f("w_out"), f("b_out")

    wq_f = (ln1_s[:, None] * wq).astype(np.float32)
    bq_f = (bq + ln1_b @ wq).astype(np.float32)
    wk_f = (ln1_s[:, None] * wk).astype(np.float32)
    bk_f = (bk + ln1_b @ wk).astype(np.float32)
    wv_f = (ln1_s[:, None] * wv).astype(np.float32)
    bv_f = (bv + ln1_b @ wv).astype(np.float32)
    w1_f = (ln2_s[:, None] * w1).astype(np.float32)
    b1_f = (b1 + ln2_b @ w1).astype(np.float32)
    wout_f = (lnf_s[:, None] * w_out).astype(np.float32)
    bout_f = (b_out + lnf_b @ w_out).astype(np.float32)

    sl = _slopes()
    qrow = np.zeros((H, S), np.float32)
    tb = np.zeros((P, H * SM), np.float32)
    s_idx = np.arange(S, dtype=np.float64)
    p_idx = np.arange(P, dtype=np.float64)
    for h in range(H):
        sgn = -1.0 if h < H // 2 else 1.0  # sign of the per-s row term
        qrow[h] = (sgn * 8.0 * sl[h % 4] * s_idx).astype(np.float32)
        for j in range(SM):
            tb[:, h * SM + j] = (-sgn * sl[h % 4] * (j * P + p_idx)).astype(
                np.float32
            )
    maskf = np.triu(np.ones((P, P), np.float32))  # keep t <= s (p <= c)
    maskb = np.tril(np.ones((P, P), np.float32))  # keep t >= s (p >= c)

    def tile3(arr):
        # [C*P, N] -> [P, C, N] contiguous (pre-tiled for SBUF layout)
        cp, n = arr.shape
        return np.ascontiguousarray(
            arr.reshape(cp // P, P, n).transpose(1, 0, 2)
        )

    common = {
        "w_in": tile3(np.asarray(inputs["w_in"], np.float32).astype(NPBF16)),
        "b_in": np.asarray(inputs["b_in"], np.float32),
        "wq": tile3(wq_f.astype(NPBF16)),
        "wk": tile3(wk_f.astype(NPBF16)),
        "wv": tile3(wv_f.astype(NPBF16)),
        "wo": tile3(np.asarray(inputs["wo"], np.float32).astype(NPBF16)),
        "bo": np.asarray(inputs["bo"], np.float32),
        "w1": tile3(w1_f.astype(NPBF16)),
        "w2": tile3(np.asarray(inputs["w2"], np.float32).astype(NPBF16)),
        "b2": np.asarray(inputs["b2"], np.float32),
        "w_out": tile3(wout_f.astype(NPBF16)),
        "b_out": bout_f,
        "bqc": np.ascontiguousarray(bq_f.reshape(DK, P).T),
        "bkc": np.ascontiguousarray(bk_f.reshape(DK, P).T),
        "b1c": np.ascontiguousarray(b1_f.reshape(FK, P).T),
        "bv": bv_f,
        "qrow": qrow.astype(NPBF16),
        "tb": tb,
        "maskf": maskf.astype(NPBF16),
        "maskb": maskb.astype(NPBF16),
        "ident": np.eye(P, dtype=NPBF16),
    }
    return common


_NC_CACHE = {}


def get_nc(gelu_mode="gelu", zb=False):
    key = (gelu_mode, zb)
    if key not in _NC_CACHE:
        _NC_CACHE[key] = build_nc(gelu_mode, zb)
    return _NC_CACHE[key]


def _all_zero_biases(common):
    names = ["b_in", "bo", "b2", "b_out", "bv"]
    if any(np.any(np.asarray(common[k], np.float32)) for k in names):
        return False
    if np.any(common["bqc"]) or np.any(common["bkc"]):
        return False
    return True


def run(inputs, trace=False):
    common = host_prep(inputs)
    x = np.asarray(inputs["x"], np.float32)
    def xtile(xi):
        # [S, D] -> xT [D, S] -> [P, DK, S] contiguous
        xt = xi.T.astype(NPBF16)
        return np.ascontiguousarray(
            xt.reshape(DK, P, S).transpose(1, 0, 2)
        )

    in_maps = [dict(common, x=xtile(x[i])) for i in range(N_CORES)]
    nc = get_nc(zb=_all_zero_biases(common))
    res = run_bass_kernel_spmd(
        nc, in_maps, core_ids=list(range(N_CORES)), trace=trace
    )
    out = np.stack([res.results[i]["out"] for i in range(N_CORES)])
    return out.astype(np.float32), res


def kernel(**inputs):
    out, _ = run(inputs, trace=False)
    return out


# revision 36
# speedup vs baseline: 1.1274x; 1.0183x over previous
"""Trainium2 Bass kernel: 1-layer transformer block w/ ALiBi bidirectional attention.

Sharding: data-parallel over batch (B=8) across 8 NeuronCores; zero collectives.

Per-core dataflow (S=1024, D=512, H=8, HD=64, FFN=2048), bf16 matmuls / fp32 PSUM:
  - Host supplies x pre-transposed (xT [d, s]); activations kept natural [s, d]
    for LayerNorm, transposed via PE (identity matmul) where matmuls need it.
  - ALiBi factorization: bias(s,t) = +-slope*(t - s) splits into a per-s term
    (folded into an augmented K=65 row of the q operand; its bf16 rounding is
    constant per softmax column so it cancels exactly in the normalization)
    and a per-t term (the per-partition ACT bias of the fused exp; scores are
    computed transposed: [t partitions, s free]).
  - Each head is half-masked (-1e9) => only the triangular half of the S x S
    score tiles is computed. Diagonal 128x128 tiles are masked by elementwise
    multiply with a 0/1 triangle.
  - q/k are projected per head ([64, 512] PSUM out) so no partition-shifting
    DMAs are needed to split heads.
  - softmax denominator r[s] comes free as an extra output row of the
    probs@V matmul (ones column appended to V); probs@V batches 4 s-chunks
    per PSUM bank; 1/r fused into the PSUM->attn_nat copy.
  - Attention head loop is software-pipelined: scores/exp run two heads
    ahead of probs@V so the PE never drains (avoids HAM re-throttle).
  - LN scale/bias of all three LNs folded into the following weight matrices
    host-side (exact algebra); LN stats batched: one Rsqrt per LN.
"""

import sys

import ml_dtypes
import numpy as np

sys.path.insert(0, "/opt/trn_rl_repo")

import concourse.bass as bass  # noqa: E402,F401
from concourse import bacc  # noqa: E402
import concourse.tile as tile  # noqa: E402
from concourse import mybir  # noqa: E402
from concourse.bass_utils import run_bass_kernel_spmd  # noqa: E402

F32 = mybir.dt.float32
BF16 = mybir.dt.bfloat16
NPBF16 = ml_dtypes.bfloat16
AF = mybir.ActivationFunctionType
OP = mybir.AluOpType

P = 128
B = 8
S = 1024
D = 512
H = 8
HD = 64
FFN = 4 * D
SM = S // P  # 8 sequence chunks
DK = D // P  # 4 feature chunks
FK = FFN // P  # 16 ffn chunks
EPS = 1e-5
N_CORES = 8


def _slopes():
    half = H // 2
    base = 24.0 ** (1.0 / half)
    return (1.0 / base ** np.arange(1, half + 1)).astype(np.float64)


def _fwd(h):
    return h < H // 2


# per (head, j) score-tile geometry for the transposed scores [t=j*128+p, s]
def _s_range(h, j):
    if _fwd(h):  # keep t <= s : s-chunks j..7
        return j * P, S - j * P
    else:  # keep t >= s : s-chunks 0..j
        return 0, (j + 1) * P


def _eoff(h, j):
    off = 0
    for jj in range(j):
        off += _s_range(h, jj)[1]
    return off


def _ewidth(h):
    return _eoff(h, SM - 1) + _s_range(h, SM - 1)[1]  # = 4608


def build_nc(gelu_mode="gelu", zb=False):
    nc = bacc.Bacc("TRN2", target_bir_lowering=False, debug=False)

    def din(name, shape, dt=F32):
        return nc.dram_tensor(name, list(shape), dt, kind="ExternalInput").ap()

    d = {}
    # all big operands pre-tiled host-side: [partition, chunk, free] so every
    # per-partition DMA read is contiguous (full DMA bandwidth)
    d["x"] = din("x", (P, DK, S), BF16)  # pre-transposed host-side
    d["w_in"] = din("w_in", (P, DK, D), BF16)
    d["b_in"] = din("b_in", (D,))
    d["wq"] = din("wq", (P, DK, D), BF16)
    d["wk"] = din("wk", (P, DK, D), BF16)
    d["wv"] = din("wv", (P, DK, D), BF16)
    d["wo"] = din("wo", (P, DK, D), BF16)
    d["bo"] = din("bo", (D,))
    d["w1"] = din("w1", (P, DK, FFN), BF16)
    d["w2"] = din("w2", (P, FK, D), BF16)
    d["b2"] = din("b2", (D,))
    d["w_out"] = din("w_out", (P, DK, D), BF16)
    d["b_out"] = din("b_out", (D,))
    d["bqc"] = din("bqc", (P, DK))
    d["bkc"] = din("bkc", (P, DK))
    d["b1c"] = din("b1c", (P, FK))
    d["bv"] = din("bv", (D,))
    d["qrow"] = din("qrow", (H, S), BF16)
    d["tb"] = din("tb", (P, H * SM))
    d["maskf"] = din("maskf", (P, P), BF16)
    d["maskb"] = din("maskb", (P, P), BF16)
    d["ident"] = din("ident", (P, P), BF16)
    d["out"] = nc.dram_tensor("out", [S, D], F32, kind="ExternalOutput").ap()

    with tile.TileContext(nc) as tc:
        _emit(nc, tc, d, gelu_mode, zb)
    nc.compile()
    return nc


def _emit(nc, tc, d, gelu_mode, zb):
    pool = tc.alloc_tile_pool

    pc = pool(name="consts", bufs=1)
    ph = pool(name="resid", bufs=3)  # tag "h": h1, attn_nat, h2, h3 rotate
    phT = pool(name="transposed", bufs=2)  # tag "hT": xT,hn1T,attnT2,hn2T,hn3T
    psm = pool(name="smalls", bufs=4)
    phn = pool(name="hn_nat", bufs=8)
    pg = pool(name="gelu", bufs=3)
    posb = pool(name="outsb", bufs=3)

    ps_mm = pool(name="ps_mm", bufs=2, space="PSUM")
    ps_acc = pool(name="ps_acc", bufs=4, space="PSUM")
    ps_tr = pool(name="ps_tr", bufs=2, space="PSUM")

    # ---- DMAs in consumption order: x, w_in first (critical path) ----
    xT = phT.tile([P, DK, S], BF16, tag="hT")
    nc.sync.dma_start(out=xT[:, 0:2, :], in_=d["x"][:, 0:2, :])
    nc.scalar.dma_start(out=xT[:, 2:4, :], in_=d["x"][:, 2:4, :])

    pwqkv = pool(name="wqkv", bufs=1)
    win_sb = pwqkv.tile([P, DK, D], BF16, tag="w_in")
    nc.scalar.dma_start(out=win_sb, in_=d["w_in"])

    # small consts on the gpsimd queue, ahead of the big weights (fast)
    ident = pc.tile([P, P], BF16, tag="ident")
    nc.gpsimd.dma_start(out=ident, in_=d["ident"])
    maskf = pc.tile([P, P], BF16, tag="maskf")
    nc.gpsimd.dma_start(out=maskf, in_=d["maskf"])
    maskb = pc.tile([P, P], BF16, tag="maskb")
    nc.gpsimd.dma_start(out=maskb, in_=d["maskb"])
    tb = pc.tile([P, H * SM], F32, tag="tb")
    nc.gpsimd.dma_start(out=tb, in_=d["tb"])
    b1c = pc.tile([P, FK], F32, tag="b1c")
    nc.gpsimd.dma_start(out=b1c, in_=d["b1c"])
    qrowc = pc.tile([1, H, S], BF16, tag="qrowc")
    nc.gpsimd.dma_start(out=qrowc, in_=d["qrow"].rearrange("h s -> (h s)").unsqueeze(0).rearrange("o (h s) -> o h s", h=H))

    epsc = pc.tile([P, 1], F32, tag="epsc")
    nc.vector.memset(epsc, EPS)

    if gelu_mode != "gelu":
        b1cs = pc.tile([P, FK], F32, tag="b1cs")
        nc.vector.tensor_scalar(
            b1cs, b1c, scalar1=1.702, scalar2=None, op0=OP.mult
        )

    def bcast(name):
        t = pc.tile([P, D], F32, tag=name + "B")
        nc.gpsimd.dma_start(out=t, in_=d[name].partition_broadcast(P))
        return t

    if not zb:
        bqc = pc.tile([P, DK], F32, tag="bqc")
        nc.gpsimd.dma_start(out=bqc, in_=d["bqc"])
        bkc = pc.tile([P, DK], F32, tag="bkc")
        nc.gpsimd.dma_start(out=bkc, in_=d["bkc"])
        binB = bcast("b_in")
        bvB = bcast("bv")
        boB = bcast("bo")
        b2B = bcast("b2")
        boutB = bcast("b_out")
    else:
        bqc = bkc = None

    # remaining weights stream in behind the first-stage ones
    wv_sb = pwqkv.tile([P, DK, D], BF16, tag="wv")
    nc.scalar.dma_start(out=wv_sb, in_=d["wv"])
    wq_sb = pwqkv.tile([P, DK, D], BF16, tag="wq")
    nc.scalar.dma_start(out=wq_sb, in_=d["wq"])
    wk_sb = pwqkv.tile([P, DK, D], BF16, tag="wk")
    nc.scalar.dma_start(out=wk_sb, in_=d["wk"])
    pwo = pool(name="wo_pool", bufs=1)
    wo_sb = pwo.tile([P, DK, D], BF16, tag="wo")
    nc.gpsimd.dma_start(out=wo_sb, in_=d["wo"])
    pwbig = pool(name="wbig", bufs=1)
    w1_sb = pwbig.tile([P, DK, FFN], BF16, tag="w1")
    nc.gpsimd.dma_start(out=w1_sb, in_=d["w1"])
    w2_sb = pwbig.tile([P, FK, D], BF16, tag="w2")
    nc.gpsimd.dma_start(out=w2_sb, in_=d["w2"])
    wout_sb = pwbig.tile([P, DK, D], BF16, tag="w_out")
    nc.gpsimd.dma_start(out=wout_sb, in_=d["w_out"])

    pqk = pool(name="qkheads", bufs=1)
    pva = pool(name="vaug", bufs=1)
    pexp = pool(name="expT", bufs=3)

    def transpose_to(dst, src, eng=0):
        # src [128,128] SBUF -> dst [128,128] (SBUF dest via PSUM bounce)
        t = ps_tr.tile([P, P], BF16, tag="tr")
        nc.tensor.transpose(t, src, ident)
        if eng == 0:
            nc.vector.tensor_copy(dst, t)
        else:
            nc.scalar.copy(dst, t)

    # h1 = x @ w_in + b_in    (natural), skewed with LN1 + v projection
    h1 = ph.tile([P, SM, D], BF16, tag="h")

    # q/k projections: combined dout-pair layout; heads split to qTa/kTa
    # tiles via SBUF->SBUF DMAs spread over two queues, one half at a time
    qTa_t = pqk.tile([65, H, S], BF16, tag="qTa", name="qTa_t")
    nc.vector.tensor_copy(qTa_t[64:65, :, :], qrowc)
    kTa_t = pqk.tile([65, H, S], BF16, tag="kTa", name="kTa_t")
    nc.vector.memset(kTa_t[64:65, :, :], 1.0)
    qTa = {h: qTa_t[:, h, :] for h in range(H)}
    kTa = {h: kTa_t[:, h, :] for h in range(H)}
    pstage = pool(name="stage", bufs=3)
    dma_q = [nc.sync, nc.scalar]
    qno = [0]

    def qk_half(half):
        sl = slice(half * 512, (half + 1) * 512)
        for dd in range(DK):  # head pair (2*dd, 2*dd+1)
            for w_sb, bc, dst in ((wq_sb, bqc, qTa), (wk_sb, bkc, kTa)):
                psq = ps_mm.tile([P, D], F32, tag="mm", name="psq")
                for dk in range(DK):
                    nc.tensor.matmul(
                        psq,
                        w_sb[:, dk, dd * P : (dd + 1) * P],
                        hn1T[:, dk, sl],
                        start=(dk == 0),
                        stop=(dk == DK - 1),
                    )
                stg = pstage.tile([P, D], BF16, tag="stg")
                if zb:
                    nc.scalar.copy(stg, psq)
                else:
                    nc.vector.tensor_scalar(
                        stg, psq, scalar1=bc[:, dd : dd + 1], scalar2=None,
                        op0=OP.add,
                    )
                dma_q[qno[0] % 2].dma_start(
                    out=dst[2 * dd][0:HD, sl], in_=stg[0:HD, :]
                )
                qno[0] += 1
                dma_q[qno[0] % 2].dma_start(
                    out=dst[2 * dd + 1][0:HD, sl], in_=stg[HD:P, :]
                )
                qno[0] += 1

    def h1_m(m):
        ps = ps_mm.tile([P, D], F32, tag="mm")
        for dk in range(DK):
            nc.tensor.matmul(
                ps,
                xT[:, dk, m * P : (m + 1) * P],
                win_sb[:, dk, :],
                start=(dk == 0),
                stop=(dk == DK - 1),
            )
        if zb:
            nc.scalar.copy(h1[:, m, :], ps)
        else:
            nc.vector.tensor_tensor(out=h1[:, m, :], in0=ps, in1=binB, op=OP.add)

    def ln_chain(src, m):
        # LayerNorm scalar chain of chunk m: produces normalized hn tile.
        # hn = (src - mean) * rstd, scale/bias folded into weights host-side
        stats = psm.tile([P, 6], F32, tag="st")
        nc.vector.bn_stats(stats, src[:, m, :])
        mv = psm.tile([P, 2], F32, tag="mv")
        nc.vector.bn_aggr(mv, stats)
        sq = psm.tile([P, 1], F32, tag="sq")
        nc.scalar.activation(sq, mv[:, 1:2], AF.Sqrt, bias=epsc)
        rstd = psm.tile([P, 1], F32, tag="rstd")
        nc.vector.reciprocal(rstd, sq)
        hn = phn.tile([P, D], BF16, tag="hn")
        nc.vector.tensor_scalar(
            hn, src[:, m, :], scalar1=mv[:, 0:1], scalar2=rstd,
            op0=OP.subtract, op1=OP.mult,
        )
        return hn

    def ln_trs(hn, dstT, m):
        for dk in range(DK):
            transpose_to(
                dstT[:, dk, m * P : (m + 1) * P],
                hn[:, dk * P : (dk + 1) * P],
                eng=dk % 2,
            )

    # hn1T = LN1(h1) transposed [d, s]; v right behind its chunk
    hn1T = phT.tile([P, DK, S], BF16, tag="hT")
    v_aug = pva.tile([P, SM, H, 65], BF16, tag="vaug")

    def v_t(t):
        psv = ps_mm.tile([P, D], F32, tag="mm", name="psv")
        for dk in range(DK):
            nc.tensor.matmul(
                psv,
                hn1T[:, dk, t * P : (t + 1) * P],
                wv_sb[:, dk, :],
                start=(dk == 0),
                stop=(dk == DK - 1),
            )
        if zb:
            nc.scalar.copy(
                v_aug[:, t, :, 0:64], psv.rearrange("p (h e) -> p h e", h=H)
            )
        else:
            nc.vector.tensor_tensor(
                out=v_aug[:, t, :, 0:64],
                in0=psv.rearrange("p (h e) -> p h e", h=H),
                in1=bvB.rearrange("p (h e) -> p h e", h=H),
                op=OP.add,
            )
        nc.vector.memset(v_aug[:, t, :, 64:65], 1.0)

    hns = {}
    for m in range(SM):
        h1_m(m)
        hns[m] = ln_chain(h1, m)
    for m in range(SM):
        ln_trs(hns.pop(m), hn1T, m)
        v_t(m)
        if m == 3:
            qk_half(0)
    qk_half(1)


    # ---- attention: software-pipelined over heads ----
    attn_nat = ph.tile([P, SM, D], BF16, tag="h", name="attn_nat")
    expTs = {}

    def stage_scores(h):
        # scores + fused exp + diagonal mask for head h -> expTs[h]
        expT = pexp.tile([P, _ewidth(h)], BF16, tag="expT", name=f"expT{h}")
        expTs[h] = expT
        for j in range(SM):
            s0, w = _s_range(h, j)
            eo = _eoff(h, j)
            off = 0
            while off < w:
                pw = min(512, w - off)
                pss = ps_mm.tile([P, pw], F32, tag="mm", name="pss")
                nc.tensor.matmul(
                    pss,
                    kTa[h][:, j * P : (j + 1) * P],
                    qTa[h][:, s0 + off : s0 + off + pw],
                    start=True,
                    stop=True,
                )
                nc.scalar.activation(
                    expT[:, eo + off : eo + off + pw],
                    pss,
                    AF.Exp,
                    bias=tb[:, h * SM + j : h * SM + j + 1],
                    scale=0.125,
                )
                off += pw
            # mask the diagonal 128x128 block (keep t<=s fwd / t>=s bwd)
            dg = eo if _fwd(h) else eo + j * P
            msk = maskf if _fwd(h) else maskb
            nc.vector.tensor_tensor(
                out=expT[:, dg : dg + P],
                in0=expT[:, dg : dg + P],
                in1=msk,
                op=OP.mult,
            )

    def stage_pv(h):
        # probs @ V with expT stationary (FWL-eligible): output comes out
        # NATURAL [s, dims+denom] - no transpose back needed. 65 rows/matmul.
        expT = expTs[h]
        for half in range(2):
            pvn = ps_acc.tile([P, 4, 66], F32, tag="acc", name=f"pvn_{h}_{half}")
            for mm in range(4):
                m = half * 4 + mm
                js = list(range(0, m + 1)) if _fwd(h) else list(range(m, SM))
                for i, j in enumerate(js):
                    s0, _w = _s_range(h, j)
                    col = _eoff(h, j) + (m * P - s0)
                    nc.tensor.matmul(
                        pvn[:, mm, 0:65],
                        expT[:, col : col + P],
                        v_aug[:, j, h, :],
                        start=(i == 0),
                        stop=(i == len(js) - 1),
                    )
            for mm in range(4):
                m = half * 4 + mm
                rinv = psm.tile([P, 1], F32, tag="rinv")
                nc.vector.reciprocal(rinv, pvn[:, mm, 64:65])
                nc.vector.tensor_scalar(
                    attn_nat[:, m, h * HD : (h + 1) * HD],
                    pvn[:, mm, 0:64],
                    scalar1=rinv,
                    scalar2=None,
                    op0=OP.mult,
                )

    stage_scores(0)
    stage_scores(1)
    for h in range(H):
        if h + 2 < H:
            stage_scores(h + 2)
        stage_pv(h)

    # attn transpose + wo + residual + LN2, pipelined per m-chunk
    attnT2 = phT.tile([P, DK, S], BF16, tag="hT")
    h2 = ph.tile([P, SM, D], BF16, tag="h")
    hn2T = phT.tile([P, DK, S], BF16, tag="hT")

    def wo_m(m):
        for dk in range(DK):
            transpose_to(
                attnT2[:, dk, m * P : (m + 1) * P],
                attn_nat[:, m, dk * P : (dk + 1) * P],
                eng=dk % 2,
            )
        ps = ps_mm.tile([P, D], F32, tag="mm", name="pswo")
        for dk in range(DK):
            nc.tensor.matmul(
                ps,
                attnT2[:, dk, m * P : (m + 1) * P],
                wo_sb[:, dk, :],
                start=(dk == 0),
                stop=(dk == DK - 1),
            )
        if zb:
            nc.vector.tensor_tensor(
                out=h2[:, m, :], in0=ps, in1=h1[:, m, :], op=OP.add
            )
        else:
            nc.vector.tensor_tensor(out=h2[:, m, :], in0=ps, in1=boB, op=OP.add)
            nc.vector.tensor_tensor(
                out=h2[:, m, :], in0=h2[:, m, :], in1=h1[:, m, :], op=OP.add
            )

    hns2 = {}
    for m in range(SM):
        wo_m(m)
        if m >= 1:
            hns2[m - 1] = ln_chain(h2, m - 1)
    hns2[SM - 1] = ln_chain(h2, SM - 1)
    for m in range(SM):
        ln_trs(hns2.pop(m), hn2T, m)

    # ---- FFN: h3 = h2 + gelu(hn2 @ w1 + b1) @ w2 + b2 ----
    # final LN + out projection run per-chunk inside the FFN tail so the
    # second half's GEMMs overlap the first half's LN3/out chain
    h3 = ph.tile([P, SM, D], BF16, tag="h")
    hn3T = phT.tile([P, DK, S], BF16, tag="hT")
    hns3 = {}
    out_view = d["out"].rearrange("(c p) n -> p c n", p=P)

    def out_m(m):
        ps = ps_mm.tile([P, D], F32, tag="mm", name="psout")
        for dk in range(DK):
            nc.tensor.matmul(
                ps,
                hn3T[:, dk, m * P : (m + 1) * P],
                wout_sb[:, dk, :],
                start=(dk == 0),
                stop=(dk == DK - 1),
            )
        osb = posb.tile([P, D], F32, tag="osb")
        if zb:
            nc.scalar.copy(osb, ps)
        else:
            nc.vector.tensor_tensor(out=osb, in0=ps, in1=boutB, op=OP.add)
        eng = (nc.sync, nc.scalar, nc.gpsimd)[m % 3]
        eng.dma_start(out=out_view[:, m, :], in_=osb)
    for half in range(2):
        accs = []
        for mm in range(4):
            accs.append(ps_acc.tile([P, D], F32, tag="acc", name=f"ff2ps{mm}"))
        for kc in range(FK):
            if half == 1 and kc == 10:
                for m in range(0, 4):
                    ln_trs(hns3.pop(m), hn3T, m)
                    out_m(m)
            ps1 = ps_mm.tile([P, 512], F32, tag="mm", name="ff1ps")
            for dk in range(DK):
                nc.tensor.matmul(
                    ps1,
                    w1_sb[:, dk, kc * P : (kc + 1) * P],
                    hn2T[:, dk, half * 512 : (half + 1) * 512],
                    start=(dk == 0),
                    stop=(dk == DK - 1),
                )
            gt = pg.tile([P, 512], BF16, tag="gt")
            if gelu_mode == "gelu":
                nc.scalar.activation(gt, ps1, AF.Gelu, bias=b1c[:, kc : kc + 1])
            else:  # CoreSim lacks Gelu: x*sigmoid(1.702x) stand-in
                sg = pg.tile([P, 512], BF16, tag="sg")
                nc.scalar.activation(
                    sg, ps1, AF.Sigmoid, bias=b1cs[:, kc : kc + 1], scale=1.702
                )
                xb = pg.tile([P, 512], BF16, tag="xb")
                nc.vector.tensor_scalar(
                    xb, ps1, scalar1=b1c[:, kc : kc + 1], scalar2=None, op0=OP.add
                )
                nc.vector.tensor_tensor(out=gt, in0=sg, in1=xb, op=OP.mult)
            for mm in range(4):
                nc.tensor.matmul(
                    accs[mm],
                    gt[:, mm * P : (mm + 1) * P],
                    w2_sb[:, kc, :],
                    start=(kc == 0),
                    stop=(kc == FK - 1),
                )
        for mm in range(4):
            m = half * 4 + mm
            if zb:
                nc.vector.tensor_tensor(
                    out=h3[:, m, :], in0=accs[mm], in1=h2[:, m, :], op=OP.add
                )
            else:
                nc.vector.tensor_tensor(
                    out=h3[:, m, :], in0=accs[mm], in1=b2B, op=OP.add
                )
                nc.vector.tensor_tensor(
                    out=h3[:, m, :], in0=h3[:, m, :], in1=h2[:, m, :], op=OP.add
                )
            hns3[m] = ln_chain(h3, m)
    for m in range(4, SM):
        ln_trs(hns3.pop(m), hn3T, m)
        out_m(m)

    for p_ in (pstage, pexp, pva, pqk, pwbig, pwo, pwqkv, ps_tr, ps_acc,
               ps_mm, posb, pg, phn, psm, phT, ph, pc):
        p_.release()


def host_prep(inputs):
    """Fold LN affine params into weights; build ALiBi helper tensors."""
    f = lambda k: np.asarray(inputs[k], dtype=np.float64)
    ln1_s, ln1_b = f("ln1_s"), f("ln1_b")
    ln2_s, ln2_b = f("ln2_s"), f("ln2_b")
    lnf_s, lnf_b = f("lnf_s"), f("lnf_b")
    wq, bq = f("wq"), f("bq")
    wk, bk = f("wk"), f("bk")
    wv, bv = f("wv"), f("bv")
    w1, b1 = f("w1"), f("b1")
    w_out, b_out = f("w_out"), f("b_out")

    wq_f = (ln1_s[:, None] * wq).astype(np.float32)
    bq_f = (bq + ln1_b @ wq).astype(np.float32)
    wk_f = (ln1_s[:, None] * wk).astype(np.float32)
    bk_f = (bk + ln1_b @ wk).astype(np.float32)
    wv_f = (ln1_s[:, None] * wv).astype(np.float32)
    bv_f = (bv + ln1_b @ wv).astype(np.float32)
    w1_f = (ln2_s[:, None] * w1).astype(np.float32)
    b1_f = (b1 + ln2_b @ w1).astype(np.float32)
    wout_f = (lnf_s[:, None] * w_out).astype(np.float32)
    bout_f = (b_out + lnf_b @ w_out).astype(np.float32)

    sl = _slopes()
    qrow = np.zeros((H, S), np.float32)
    tb = np.zeros((P, H * SM), np.float32)
    s_idx = np.arange(S, dtype=np.float64)
    p_idx = np.arange(P, dtype=np.float64)
    for h in range(H):
        sgn = -1.0 if h < H // 2 else 1.0  # sign of the per-s row term
        qrow[h] = (sgn * 8.0 * sl[h % 4] * s_idx).astype(np.float32)
        for j in range(SM):
            tb[:, h * SM + j] = (-sgn * sl[h % 4] * (j * P + p_idx)).astype(
                np.float32
            )
    maskf = np.triu(np.ones((P, P), np.float32))  # keep t <= s (p <= c)
    maskb = np.tril(np.ones((P, P), np.float32))  # keep t >= s (p >= c)

    def tile3(arr):
        # [C*P, N] -> [P, C, N] contiguous (pre-tiled for SBUF layout)
        cp, n = arr.shape
        return np.ascontiguousarray(
            arr.reshape(cp // P, P, n).transpose(1, 0, 2)
        )

    common = {
        "w_in": tile3(np.asarray(inputs["w_in"], np.float32).astype(NPBF16)),
        "b_in": np.asarray(inputs["b_in"], np.float32),
        "wq": tile3(wq_f.astype(NPBF16)),
        "wk": tile3(wk_f.astype(NPBF16)),
        "wv": tile3(wv_f.astype(NPBF16)),
        "wo": tile3(np.asarray(inputs["wo"], np.float32).astype(NPBF16)),
        "bo": np.asarray(inputs["bo"], np.float32),
        "w1": tile3(w1_f.astype(NPBF16)),
        "w2": tile3(np.asarray(inputs["w2"], np.float32).astype(NPBF16)),
        "b2": np.asarray(inputs["b2"], np.float32),
        "w_out": tile3(wout_f.astype(NPBF16)),
        "b_out": bout_f,
        "bqc": np.ascontiguousarray(bq_f.reshape(DK, P).T),
        "bkc": np.ascontiguousarray(bk_f.reshape(DK, P).T),
        "b1c": np.ascontiguousarray(b1_f.reshape(FK, P).T),
        "bv": bv_f,
        "qrow": qrow.astype(NPBF16),
        "tb": tb,
        "maskf": maskf.astype(NPBF16),
        "maskb": maskb.astype(NPBF16),
        "ident": np.eye(P, dtype=NPBF16),
    }
    return common


_NC_CACHE = {}


def get_nc(gelu_mode="gelu", zb=False):
    key = (gelu_mode, zb)
    if key not in _NC_CACHE:
        _NC_CACHE[key] = build_nc(gelu_mode, zb)
    return _NC_CACHE[key]


def _all_zero_biases(common):
    names = ["b_in", "bo", "b2", "b_out", "bv"]
    if any(np.any(np.asarray(common[k], np.float32)) for k in names):
        return False
    if np.any(common["bqc"]) or np.any(common["bkc"]):
        return False
    return True


def run(inputs, trace=False):
    common = host_prep(inputs)
    x = np.asarray(inputs["x"], np.float32)
    def xtile(xi):
        # [S, D] -> xT [D, S] -> [P, DK, S] contiguous
        xt = xi.T.astype(NPBF16)
        return np.ascontiguousarray(
            xt.reshape(DK, P, S).transpose(1, 0, 2)
        )

    in_maps = [dict(common, x=xtile(x[i])) for i in range(N_CORES)]
    nc = get_nc(zb=_all_zero_biases(common))
    res = run_bass_kernel_spmd(
        nc, in_maps, core_ids=list(range(N_CORES)), trace=trace
    )
    out = np.stack([res.results[i]["out"] for i in range(N_CORES)])
    return out.astype(np.float32), res


def kernel(**inputs):
    out, _ = run(inputs, trace=False)
    return out
